# revision 14
# baseline (speedup 1.0000x reference)
"""Trainium2 Bass kernel for nn_Conv2D_26164940767465.

Per-(channel, filter) VALID 2D cross-correlation with NO channel reduction:
  out[b, ho, c, f, wo] = sum_{i,j} int(x[b, ho+i, wo+j, c]) * int(k[i,j,c,f])

Shapes: x (4,224,224,16) f32 integer-valued [0,256); k (5,5,16,32) f32
integer-valued [-8,8). Output (4,220,16,32,220) f32.

Exactness: x <= 255 and |k| <= 8 are exactly representable in bf16; products
(<= 2040) and 25-tap sums (|.| <= 51000 < 2^24) are exact in the fp32 PSUM
accumulator. So a bf16 tensor-engine matmul reproduces the int32 reference
bit-exactly.

Strategy (8 cores): shard (batch 4) x (output-row halves 2). Per core the
input lives in SBUF once, as a j-shifted channel-major buffer XSJ on a padded
128-partition layout: partition 32*g + c_l*5 + j holds row-major image rows of
channel c = 4*g + c_l shifted left by j. For every pair of output rows, the
four channel groups g run CONCURRENTLY as 4 row-tiled matmuls (K=20 each,
tile_position=(32g,0)) into 4 PSUM banks, accumulating the 5 kernel rows i
via rhs free-offset (r+i)*WO — no im2col materialization at all. PSUM tiles
are evacuated by vector/scalar engines and written out in 10-row chunks whose
per-partition DRAM runs are fully contiguous (out layout [C, F, rows, wo]).
"""

import os
import sys

if "/opt/trn_rl_repo" not in sys.path:
    sys.path.insert(0, "/opt/trn_rl_repo")

import numpy as np
import ml_dtypes

BF16 = np.dtype(ml_dtypes.bfloat16)

# Problem constants (hardcoded per harness contract).
B, H, W, C = 4, 224, 224, 16
KH, KW, F = 5, 5, 32
HO, WO = H - KH + 1, W - KW + 1          # 220, 220
NCORES = 8
HALF = HO // 2                            # 110 output rows per core
HIN = HALF + KH - 1                       # 114 input rows per core
CG = 4                                    # channels per group
NG = C // CG                              # 4 groups
KJ = CG * KW                              # 20 contraction rows per group
MP = CG * F                               # 128 output partitions
ROWS_PER_MM = 2
NFREE = ROWS_PER_MM * WO                  # 440
ROWS_PER_CHUNK = 10                       # rows staged per output DMA
MM_PER_CHUNK = ROWS_PER_CHUNK // ROWS_PER_MM   # 5 row-pairs
NCHUNK = HALF // ROWS_PER_CHUNK           # 11

_PROGRAM = None


def _build_program():
    import concourse.bacc as bacc
    import concourse.mybir as mybir
    import concourse.tile as tile

    nc = bacc.Bacc("TRN2", target_bir_lowering=False, debug=False,
                   num_devices=NCORES)

    xsj_d = nc.dram_tensor("xsj", [NG * KJ, HIN * WO], mybir.dt.bfloat16,
                           kind="ExternalInput")
    wt_d = nc.dram_tensor("wt", [128, KH * MP], mybir.dt.bfloat16,
                          kind="ExternalInput")
    # [C, F, rows, wo] layout: each output partition (c_l, f) owns a fully
    # contiguous DRAM run per chunk. Host transposes back on assembly.
    out_d = nc.dram_tensor("out", [C, F, HALF, WO], mybir.dt.float32,
                           kind="ExternalOutput")

    xsj_ap = xsj_d.ap()
    wt_ap = wt_d.ap()
    out_ap = out_d.ap()

    with tile.TileContext(nc) as tc:
        with (
            tc.tile_pool(name="wpool", bufs=1) as wpool,
            tc.tile_pool(name="xpool", bufs=1) as xpool,
            tc.tile_pool(name="spool", bufs=3) as spool,
            tc.tile_pool(name="psum", bufs=2, space="PSUM") as pspool,
        ):
            # Inputs on the Scalar HWDGE queue; outputs on Sync, so the big
            # input load never sits in front of output chunks in a FIFO.
            wt_t = wpool.tile([128, KH * MP], mybir.dt.bfloat16)
            nc.scalar.dma_start(wt_t[:], wt_ap)
            # Two half-range input tiles with a 14-row halo: chunks 0-4 read
            # XA (rows 0-63), chunks 5-10 read XB (rows 50-113). Tile deps
            # are whole-tile for multi-writer tiles, so splitting lets the
            # first matmuls start after ~2.3 MB instead of the full input.
            XB0 = 50
            XROWS = 64
            xa_t = xpool.tile([128, XROWS * WO], mybir.dt.bfloat16,
                              name="xa_t")
            xb_t = xpool.tile([128, XROWS * WO], mybir.dt.bfloat16,
                              name="xb_t")
            for g in range(NG):
                nc.scalar.dma_start(
                    xa_t[32 * g:32 * g + KJ, :],
                    xsj_ap[KJ * g:KJ * (g + 1), 0:XROWS * WO])
            for g in range(NG):
                nc.scalar.dma_start(
                    xb_t[32 * g:32 * g + KJ, :],
                    xsj_ap[KJ * g:KJ * (g + 1), XB0 * WO:(XB0 + XROWS) * WO])

            for ch in range(NCHUNK):
                last = ch == NCHUNK - 1
                if not last:
                    stages = [
                        spool.tile([MP, ROWS_PER_CHUNK * WO],
                                   mybir.dt.float32,
                                   tag=f"stage{g}", name=f"stage{g}")
                        for g in range(NG)
                    ]
                for t in range(MM_PER_CHUNK):
                    r = ch * ROWS_PER_CHUNK + t * ROWS_PER_MM
                    x_t, rbase = (xa_t, 0) if ch < 5 else (xb_t, XB0)
                    pss = [
                        pspool.tile([MP, NFREE], mybir.dt.float32,
                                    tag=f"ps{g}", name=f"ps{g}")
                        for g in range(NG)
                    ]
                    for i in range(KH):
                        off = (r + i - rbase) * WO
                        for g in range(NG):
                            p0 = 32 * g
                            nc.tensor.matmul(
                                pss[g][:],
                                wt_t[p0:p0 + KJ, i * MP:(i + 1) * MP],
                                x_t[p0:p0 + KJ, off:off + NFREE],
                                start=(i == 0), stop=(i == KH - 1),
                                tile_position=(p0, 0),
                            )
                    for g in range(NG):
                        if last:
                            # Final chunk: evacuate + DMA per row-pair so
                            # only ~0.9 MB remains after the last copy.
                            st = spool.tile([MP, NFREE], mybir.dt.float32,
                                            tag=f"lstage{g}",
                                            name=f"lstage{g}")
                            if (t * NG + g) % 2 == 0:
                                nc.vector.tensor_copy(st[:], pss[g][:])
                            else:
                                nc.scalar.copy(st[:], pss[g][:])
                            dram_slab = out_ap[
                                g * CG:(g + 1) * CG, :,
                                r:r + ROWS_PER_MM, :,
                            ].rearrange("c f r w -> (c f) r w")
                            nc.sync.dma_start(
                                dram_slab,
                                st[:].rearrange("p (r w) -> p r w", w=WO),
                            )
                        else:
                            dst = stages[g][:, t * NFREE:(t + 1) * NFREE]
                            if (t * NG + g) % 2 == 0:
                                nc.vector.tensor_copy(dst, pss[g][:])
                            else:
                                nc.scalar.copy(dst, pss[g][:])
                if not last:
                    for g in range(NG):
                        dram_slab = out_ap[
                            g * CG:(g + 1) * CG, :,
                            ch * ROWS_PER_CHUNK:(ch + 1) * ROWS_PER_CHUNK, :,
                        ].rearrange("c f r w -> (c f) r w")
                        nc.sync.dma_start(
                            dram_slab,
                            stages[g][:].rearrange("p (r w) -> p r w", w=WO),
                        )

    nc.compile()
    return nc


def _get_program():
    global _PROGRAM
    if _PROGRAM is None:
        _PROGRAM = _build_program()
    return _PROGRAM


def _host_pack(x, k):
    """Build per-core XSJ tensors and the shared per-tap weights (bf16)."""
    x_bf = np.ascontiguousarray(x.astype(BF16))
    k_bf = k.astype(BF16)

    xsj_all = []
    for m in range(NCORES):
        b, half = m // 2, m % 2
        r0 = half * HALF
        # Deinterleave once: [C, 114, 224] channel-major rows.
        xc = np.ascontiguousarray(x_bf[b, r0:r0 + HIN].transpose(2, 0, 1))
        xp = np.empty((NG * KJ, HIN, WO), dtype=BF16)
        for c in range(C):
            g, cl = c // CG, c % CG
            base = KJ * g + cl * KW
            for j in range(KW):
                xp[base + j] = xc[c, :, j:j + WO]
        xsj_all.append(xp.reshape(NG * KJ, HIN * WO))

    wt = np.zeros((128, KH, MP), dtype=BF16)
    for c in range(C):
        g, cl = c // CG, c % CG
        base = 32 * g + cl * KW
        for j in range(KW):
            for i in range(KH):
                wt[base + j, i, cl * F:(cl + 1) * F] = k_bf[i, j, c, :]
    wt = np.ascontiguousarray(wt.reshape(128, KH * MP))
    return xsj_all, wt


LAST_EXEC_TIME_NS = None


def kernel(**inputs):
    from concourse.bass_utils import run_bass_kernel_spmd

    global LAST_EXEC_TIME_NS
    x = np.asarray(inputs["inputs"])
    k = np.asarray(inputs["kernel"])
    assert x.shape == (B, H, W, C) and k.shape == (KH, KW, C, F)

    nc = _get_program()
    xsj_all, wt = _host_pack(x, k)
    in_maps = [{"xsj": xsj_all[m], "wt": wt} for m in range(NCORES)]

    trace = os.environ.get("CONV_TRACE", "") == "1"
    kwargs = {}
    if trace:
        kwargs["trace"] = True
        tdir = os.environ.get("CONV_TRACE_DIR")
        if tdir:
            kwargs["tmpdir"] = tdir

    res = run_bass_kernel_spmd(nc, in_maps, list(range(NCORES)), **kwargs)
    LAST_EXEC_TIME_NS = res.exec_time_ns

    full = np.empty((B, HO, C, F, WO), dtype=np.float32)
    for m in range(NCORES):
        b, half = m // 2, m % 2
        # device layout [C, F, rows, WO] -> reference layout [rows, C, F, WO]
        full[b, half * HALF:(half + 1) * HALF] = \
            res.results[m]["out"].transpose(2, 0, 1, 3)
    return full


# revision 18
# speedup vs baseline: 1.4193x; 1.4193x over previous
"""Trainium2 Bass kernel for nn_Conv2D_26164940767465.

Per-(channel, filter) VALID 2D cross-correlation with NO channel reduction:
  out[b, ho, c, f, wo] = sum_{i,j} int(x[b, ho+i, wo+j, c]) * int(k[i,j,c,f])

Shapes: x (4,224,224,16) f32 integer-valued [0,256); k (5,5,16,32) f32
integer-valued [-8,8). Output (4,220,16,32,220) f32.

Exactness: x <= 255 and |k| <= 8 are exactly representable in bf16; products
(<= 2040) and 25-tap sums (|.| <= 51000 < 2^24) are exact in the fp32 PSUM
accumulator. So a bf16 tensor-engine matmul reproduces the int32 reference
bit-exactly.

Strategy (8 cores): shard (batch 4) x (output-row halves 2). Per core the
input lives in SBUF once, as a j-shifted channel-major buffer XSJ on a padded
128-partition layout: partition 32*g + c_l*5 + j holds row-major image rows of
channel c = 4*g + c_l shifted left by j. For every pair of output rows, the
four channel groups g run CONCURRENTLY as 4 row-tiled matmuls (K=20 each,
tile_position=(32g,0)) into 4 PSUM banks, accumulating the 5 kernel rows i
via rhs free-offset (r+i)*WO — no im2col materialization at all. PSUM tiles
are evacuated by vector/scalar engines and written out in 10-row chunks whose
per-partition DRAM runs are fully contiguous (out layout [C, F, rows, wo]).
"""

import os
import sys

if "/opt/trn_rl_repo" not in sys.path:
    sys.path.insert(0, "/opt/trn_rl_repo")

import numpy as np
import ml_dtypes

BF16 = np.dtype(ml_dtypes.bfloat16)

# Problem constants (hardcoded per harness contract).
B, H, W, C = 4, 224, 224, 16
KH, KW, F = 5, 5, 32
HO, WO = H - KH + 1, W - KW + 1          # 220, 220
NCORES = 8
HALF = HO // 2                            # 110 output rows per core
HIN = HALF + KH - 1                       # 114 input rows per core
CG = 4                                    # channels per group
NG = C // CG                              # 4 groups
KJ = CG * KW                              # 20 contraction rows per group
MP = CG * F                               # 128 output partitions
ROWS_PER_MM = 2
NFREE = ROWS_PER_MM * WO                  # 440
ROWS_PER_CHUNK = 10                       # rows staged per output DMA
MM_PER_CHUNK = ROWS_PER_CHUNK // ROWS_PER_MM   # 5 row-pairs
NCHUNK = HALF // ROWS_PER_CHUNK           # 11

_PROGRAM = None


def _build_program():
    import concourse.bacc as bacc
    import concourse.mybir as mybir
    import concourse.tile as tile

    nc = bacc.Bacc("TRN2", target_bir_lowering=False, debug=False,
                   num_devices=NCORES)

    xsj_d = nc.dram_tensor("xsj", [128, HIN * WO], mybir.dt.bfloat16,
                           kind="ExternalInput")
    wt_d = nc.dram_tensor("wt", [128, KH * MP], mybir.dt.bfloat16,
                          kind="ExternalInput")
    # [C, F, rows, wo] layout: each output partition (c_l, f) owns a fully
    # contiguous DRAM run per chunk. Host transposes back on assembly.
    out_d = nc.dram_tensor("out", [C, F, HALF, WO], mybir.dt.float32,
                           kind="ExternalOutput")

    xsj_ap = xsj_d.ap()
    wt_ap = wt_d.ap()
    out_ap = out_d.ap()

    with tile.TileContext(nc) as tc:
        with (
            tc.tile_pool(name="wpool", bufs=1) as wpool,
            tc.tile_pool(name="xpool", bufs=1) as xpool,
            tc.tile_pool(name="spool", bufs=3) as spool,
            tc.tile_pool(name="psum", bufs=2, space="PSUM") as pspool,
        ):
            # Inputs on the Scalar HWDGE queue; outputs on Sync, so the big
            # input load never sits in front of output chunks in a FIFO.
            wt_t = wpool.tile([128, KH * MP], mybir.dt.bfloat16)
            nc.scalar.dma_start(wt_t[:], wt_ap)
            # Two half-range input tiles with a 14-row halo: chunks 0-4 read
            # XA (rows 0-63), chunks 5-10 read XB (rows 50-113). Tile deps
            # are whole-tile for multi-writer tiles, so splitting lets the
            # first matmuls start after ~2.3 MB instead of the full input.
            XB0 = 50
            XROWS = 64
            xa_t = xpool.tile([128, XROWS * WO], mybir.dt.bfloat16,
                              name="xa_t")
            xb_t = xpool.tile([128, XROWS * WO], mybir.dt.bfloat16,
                              name="xb_t")
            # One plain 128-partition DMA per half-tile (padded layout), so
            # every channel group's matmuls share identical input deps —
            # per-g loads made the scheduler split the 4-way concurrent
            # groups into 2+2 and doubled PE time.
            nc.scalar.dma_start(xa_t[:], xsj_ap[:, 0:XROWS * WO])
            nc.scalar.dma_start(xb_t[:],
                                xsj_ap[:, XB0 * WO:(XB0 + XROWS) * WO])

            for ch in range(NCHUNK):
                last = ch == NCHUNK - 1
                if not last:
                    stages = [
                        spool.tile([MP, ROWS_PER_CHUNK * WO],
                                   mybir.dt.float32,
                                   tag=f"stage{g}", name=f"stage{g}")
                        for g in range(NG)
                    ]
                for t in range(MM_PER_CHUNK):
                    r = ch * ROWS_PER_CHUNK + t * ROWS_PER_MM
                    x_t, rbase = (xa_t, 0) if ch < 5 else (xb_t, XB0)
                    pss = [
                        pspool.tile([MP, NFREE], mybir.dt.float32,
                                    tag=f"ps{g}", name=f"ps{g}")
                        for g in range(NG)
                    ]
                    for i in range(KH):
                        off = (r + i - rbase) * WO
                        for g in range(NG):
                            p0 = 32 * g
                            nc.tensor.matmul(
                                pss[g][:],
                                wt_t[p0:p0 + KJ, i * MP:(i + 1) * MP],
                                x_t[p0:p0 + KJ, off:off + NFREE],
                                start=(i == 0), stop=(i == KH - 1),
                                tile_position=(p0, 0),
                            )
                    for g in range(NG):
                        if last:
                            # Final chunk: evacuate + DMA per row-pair so
                            # only ~0.9 MB remains after the last copy.
                            st = spool.tile([MP, NFREE], mybir.dt.float32,
                                            tag=f"lstage{g}",
                                            name=f"lstage{g}")
                            if (t * NG + g) % 2 == 0:
                                nc.vector.tensor_copy(st[:], pss[g][:])
                            else:
                                nc.scalar.copy(st[:], pss[g][:])
                            dram_slab = out_ap[
                                g * CG:(g + 1) * CG, :,
                                r:r + ROWS_PER_MM, :,
                            ].rearrange("c f r w -> (c f) r w")
                            nc.sync.dma_start(
                                dram_slab,
                                st[:].rearrange("p (r w) -> p r w", w=WO),
                            )
                        else:
                            dst = stages[g][:, t * NFREE:(t + 1) * NFREE]
                            if (t * NG + g) % 2 == 0:
                                nc.vector.tensor_copy(dst, pss[g][:])
                            else:
                                nc.scalar.copy(dst, pss[g][:])
                if not last:
                    for g in range(NG):
                        dram_slab = out_ap[
                            g * CG:(g + 1) * CG, :,
                            ch * ROWS_PER_CHUNK:(ch + 1) * ROWS_PER_CHUNK, :,
                        ].rearrange("c f r w -> (c f) r w")
                        nc.sync.dma_start(
                            dram_slab,
                            stages[g][:].rearrange("p (r w) -> p r w", w=WO),
                        )

    nc.compile()
    return nc


def _get_program():
    global _PROGRAM
    if _PROGRAM is None:
        _PROGRAM = _build_program()
    return _PROGRAM


def _host_pack(x, k):
    """Build per-core XSJ tensors and the shared per-tap weights (bf16)."""
    x_bf = np.ascontiguousarray(x.astype(BF16))
    k_bf = k.astype(BF16)

    xsj_all = []
    for m in range(NCORES):
        b, half = m // 2, m % 2
        r0 = half * HALF
        # Deinterleave once: [C, 114, 224] channel-major rows.
        xc = np.ascontiguousarray(x_bf[b, r0:r0 + HIN].transpose(2, 0, 1))
        xp = np.zeros((128, HIN, WO), dtype=BF16)
        for c in range(C):
            g, cl = c // CG, c % CG
            base = 32 * g + cl * KW
            for j in range(KW):
                xp[base + j] = xc[c, :, j:j + WO]
        xsj_all.append(xp.reshape(128, HIN * WO))

    wt = np.zeros((128, KH, MP), dtype=BF16)
    for c in range(C):
        g, cl = c // CG, c % CG
        base = 32 * g + cl * KW
        for j in range(KW):
            for i in range(KH):
                wt[base + j, i, cl * F:(cl + 1) * F] = k_bf[i, j, c, :]
    wt = np.ascontiguousarray(wt.reshape(128, KH * MP))
    return xsj_all, wt


LAST_EXEC_TIME_NS = None


def kernel(**inputs):
    from concourse.bass_utils import run_bass_kernel_spmd

    global LAST_EXEC_TIME_NS
    x = np.asarray(inputs["inputs"])
    k = np.asarray(inputs["kernel"])
    assert x.shape == (B, H, W, C) and k.shape == (KH, KW, C, F)

    nc = _get_program()
    xsj_all, wt = _host_pack(x, k)
    in_maps = [{"xsj": xsj_all[m], "wt": wt} for m in range(NCORES)]

    trace = os.environ.get("CONV_TRACE", "") == "1"
    kwargs = {}
    if trace:
        kwargs["trace"] = True
        tdir = os.environ.get("CONV_TRACE_DIR")
        if tdir:
            kwargs["tmpdir"] = tdir

    res = run_bass_kernel_spmd(nc, in_maps, list(range(NCORES)), **kwargs)
    LAST_EXEC_TIME_NS = res.exec_time_ns

    full = np.empty((B, HO, C, F, WO), dtype=np.float32)
    for m in range(NCORES):
        b, half = m // 2, m % 2
        # device layout [C, F, rows, WO] -> reference layout [rows, C, F, WO]
        full[b, half * HALF:(half + 1) * HALF] = \
            res.results[m]["out"].transpose(2, 0, 1, 3)
    return full
